# revision 3
# baseline (speedup 1.0000x reference)
"""MoE (8 experts, top-2) Trainium2 kernel.

Strategy (per spec sharding_hint): expert parallelism. The host computes the
(cheap) router — logits, softmax, top-2, renormalized combine weights — and
dispatches each token to the cores owning its two experts ("all-to-all token
dispatch by top-k expert id" done at the sharding step, since kernel() holds
the full inputs host-side). Core e runs the expert-e FFN over its gathered
tokens, capacity-padded so all 8 cores run one SPMD program:

    Y = W2[e]^T @ gelu(W1[e]^T @ XT + b1[e])         (feature-major layouts)

Everything is bf16 on the PE (1 cycle/row) with fp32 PSUM accumulation.
Both weight matrices stay resident in SBUF for the whole kernel; the
gelu intermediate HT lives in SBUF half-buffers (never round-trips to
DRAM). Per 512-token tile: stage 1 fills HT (32 i-chunks x 8 k-matmuls),
stage 2 accumulates all 32 i-chunks into 8 output psum banks and drains
Y to DRAM. The host then scatter-adds  (Y + b2[e]) * combine  into the
full output.
"""

import os
import sys

import numpy as np

for _p in ("/opt/trn_rl_repo", "/root/.axon_site/_ro/trn_rl_repo"):
    if os.path.isdir(_p) and _p not in sys.path:
        sys.path.insert(0, _p)

NUM_EXPERTS = 8
TOP_K = 2
B, S, H, I = 4, 4096, 1024, 4096
T = B * S
P = 128
NT = 512           # max token tile = moving free dim (fp32 psum bank limit)
C_DEFAULT = 4352   # capacity per expert (seed-0 max count 4302), mult of 256

KH = H // P        # 8 contraction chunks for stage 1
KI = I // P        # 32 contraction chunks for stage 2
OB = H // P        # 8 output chunks for stage 2
HB = KI // 2       # 16 ht chunks per SBUF half-buffer

_built = {}        # (C, reps) -> nc


def _token_tiles(C):
    """Split C into tiles of 512 plus at most one trailing 256."""
    assert C % 256 == 0
    tiles, off = [], 0
    while C - off >= 512:
        tiles.append((off, 512))
        off += 512
    if C - off:
        tiles.append((off, 256))
        off = C
    return tiles


def _build(C, reps=1):
    import concourse.bacc as bacc
    import concourse.mybir as mybir
    import concourse.tile as tile
    from concourse._compat import get_trn_type

    f32 = mybir.dt.float32
    bf16 = mybir.dt.bfloat16
    GELU = mybir.ActivationFunctionType.Gelu

    nc = bacc.Bacc(
        get_trn_type() or "TRN2",
        target_bir_lowering=False,
        debug=False,
        enable_asserts=False,
    )
    xt = nc.dram_tensor("xt", [H, C], bf16, kind="ExternalInput").ap()
    w1 = nc.dram_tensor("w1", [H, I], bf16, kind="ExternalInput").ap()
    b1 = nc.dram_tensor("b1", [I], f32, kind="ExternalInput").ap()
    w2 = nc.dram_tensor("w2", [I, H], bf16, kind="ExternalInput").ap()
    y = nc.dram_tensor("y", [H, C], f32, kind="ExternalOutput").ap()

    tiles = _token_tiles(C)

    with tile.TileContext(nc) as tc:
        with (
            tc.tile_pool(name="bias", bufs=1) as bpool,
            tc.tile_pool(name="w1p", bufs=1) as w1p,
            tc.tile_pool(name="w2p", bufs=1) as w2p,
            tc.tile_pool(name="xp", bufs=3) as xp,
            tc.tile_pool(name="htp", bufs=2) as htp,
            tc.tile_pool(name="yp", bufs=4) as yp,
            tc.tile_pool(name="psp", bufs=8, space="PSUM") as psp,
        ):
            b1sb = bpool.tile([P, KI], f32)
            nc.sync.dma_start(b1sb[:], b1.rearrange("(ib p) -> p ib", p=P))
            w1r = w1.rearrange("(ko p) i -> p ko i", p=P)
            w2r = w2.rearrange("(ko p) o -> p ko o", p=P)

            for rep in range(reps):
                w1sb = w1p.tile([P, KH, I], bf16, tag="w1", name=f"w1_{rep}")
                w2sb = w2p.tile([P, KI, H], bf16, tag="w2", name=f"w2_{rep}")

                def _load_x(t, toff, tsz, ways, rep=rep):
                    xst = xp.tile([P, KH, tsz], bf16, tag="x",
                                  name=f"x_{rep}_{t}")
                    kw = KH // ways
                    for s in range(ways):
                        nc.sync.dma_start(
                            xst[:, s * kw:(s + 1) * kw],
                            xt[s * kw * P:(s + 1) * kw * P,
                               toff:toff + tsz].rearrange(
                                "(ko p) n -> p ko n", p=P),
                        )
                    return xst

                # Lead-in: x(t0) split 8 ways and w1's first i-chunk split in
                # two, so the PE's first matmul waits on ~1.25MB spread over
                # many queues; then the rest of w1 (i-column order, matching
                # stage-1 consumption) and w2 (k-chunk order for stage 2).
                xs0 = _load_x(0, tiles[0][0], tiles[0][1], ways=8)
                for lo, hi in ((0, 64), (64, 128), (128, 256), (256, 512),
                               (512, 1024), (1024, 1536), (1536, 2048),
                               (2048, 2560), (2560, 3072), (3072, 3584),
                               (3584, 4096)):
                    nc.sync.dma_start(w1sb[:, :, lo:hi], w1r[:, :, lo:hi])
                cw = 4
                for c in range(KI // cw):
                    nc.sync.dma_start(
                        w2sb[:, c * cw:(c + 1) * cw],
                        w2r[:, c * cw:(c + 1) * cw],
                    )

                for t, (toff, tsz) in enumerate(tiles):
                    xs = xs0 if t == 0 else _load_x(t, toff, tsz, ways=2)

                    # -- Stage 1: HT[i,t] = gelu(W1^T XT + b1), 2 half-bufs
                    hts = []
                    for hb in range(2):
                        htt = htp.tile([P, HB, tsz], bf16, tag="ht",
                                       name=f"ht_{rep}_{t}_{hb}")
                        for ibl in range(HB):
                            ib = hb * HB + ibl
                            ps = psp.tile([P, tsz], f32, tag="ps",
                                          name=f"ps1_{rep}_{t}_{ib}")
                            for k in range(KH):
                                nc.tensor.matmul(
                                    ps[:],
                                    lhsT=w1sb[:, k, ib * P:(ib + 1) * P],
                                    rhs=xs[:, k],
                                    start=(k == 0),
                                    stop=(k == KH - 1),
                                )
                            nc.scalar.activation(
                                htt[:, ibl], ps[:], GELU,
                                bias=b1sb[:, ib:ib + 1],
                            )
                        hts.append(htt)

                    # -- Stage 2: Y[o,t] = sum_i W2[i,o] HT[i,t], all 32
                    # i-chunks accumulated in 8 psum banks (k-outer order so
                    # each ht chunk is consumed once, freeing it early).
                    pss = [psp.tile([P, tsz], f32, tag="ps",
                                    name=f"ps2_{rep}_{t}_{ob}")
                           for ob in range(OB)]
                    for k in range(KI):
                        hb, kk = divmod(k, HB)
                        for ob in range(OB):
                            nc.tensor.matmul(
                                pss[ob][:],
                                lhsT=w2sb[:, k, ob * P:(ob + 1) * P],
                                rhs=hts[hb][:, kk],
                                start=(k == 0),
                                stop=(k == KI - 1),
                            )
                    for ob in range(OB):
                        ys = yp.tile([P, tsz], f32, tag="y",
                                     name=f"y_{rep}_{t}_{ob}")
                        nc.vector.tensor_copy(ys[:], pss[ob][:])
                        nc.sync.dma_start(
                            y[ob * P:(ob + 1) * P, toff:toff + tsz], ys[:]
                        )
    nc.finalize()
    return nc


def _routing(hidden, router_w, router_b):
    """Top-2 routing, bit-matching the jax reference on CPU."""
    import jax
    import jax.numpy as jnp

    cpu = jax.local_devices(backend="cpu")[0]
    with jax.default_device(cpu):
        logits = jnp.einsum("bsh,he->bse", jnp.asarray(hidden),
                            jnp.asarray(router_w)) + jnp.asarray(router_b)
        probs = jax.nn.softmax(logits, axis=-1)
        tkp, tki = jax.lax.top_k(probs, TOP_K)
        tkp = tkp / jnp.sum(tkp, axis=-1, keepdims=True)
        tkp_np = np.asarray(tkp).reshape(T, TOP_K)
        tki_np = np.asarray(tki).reshape(T, TOP_K)
    return tkp_np, tki_np


def _prepare(hidden_states, w1, b1, w2, b2, router_w, router_b):
    """Host-side routing + dispatch: per-core in_maps and scatter metadata."""
    hidden_states = np.ascontiguousarray(hidden_states, dtype=np.float32)
    w1 = np.ascontiguousarray(w1, dtype=np.float32)
    b1 = np.ascontiguousarray(b1, dtype=np.float32)
    w2 = np.ascontiguousarray(w2, dtype=np.float32)
    b2 = np.ascontiguousarray(b2, dtype=np.float32)

    import ml_dtypes

    bf = ml_dtypes.bfloat16
    w1_bf = w1.astype(bf)
    w2_bf = w2.astype(bf)
    tkp, tki = _routing(hidden_states, router_w, router_b)
    x = hidden_states.reshape(T, H)

    idx_e, prob_e = [], []
    for e in range(NUM_EXPERTS):
        hit = tki == e                       # [T, 2] bool
        idx = np.nonzero(hit.any(axis=1))[0]
        pe = np.where(hit[idx, 0], tkp[idx, 0], tkp[idx, 1]).astype(np.float32)
        idx_e.append(idx)
        prob_e.append(pe)

    maxn = max(len(ix) for ix in idx_e)
    C = C_DEFAULT if maxn <= C_DEFAULT else ((maxn + 255) // 256) * 256

    in_maps = []
    for e in range(NUM_EXPERTS):
        ix = idx_e[e]
        xt = np.zeros((H, C), dtype=bf)
        xt[:, :len(ix)] = x[ix].T.astype(bf)
        in_maps.append({
            "xt": xt,
            "w1": w1_bf[e],
            "b1": b1[e],
            "w2": w2_bf[e],
        })
    return in_maps, C, idx_e, prob_e, b2


def kernel(hidden_states, w1, b1, w2, b2, router_w, router_b):
    from concourse import bass_utils

    in_maps, C, idx_e, prob_e, b2 = _prepare(
        hidden_states, w1, b1, w2, b2, router_w, router_b
    )
    if (C, 1) not in _built:
        _built[(C, 1)] = _build(C)
    nc = _built[(C, 1)]

    res = bass_utils.run_bass_kernel_spmd(
        nc, in_maps, core_ids=list(range(NUM_EXPERTS))
    ).results

    out = np.zeros((T, H), dtype=np.float32)
    for e in range(NUM_EXPERTS):
        ix = idx_e[e]
        ye = res[e]["y"][:, :len(ix)].T
        out[ix] += (ye + b2[e]) * prob_e[e][:, None]
    return out.reshape(B, S, H)


# revision 4
# speedup vs baseline: 1.3188x; 1.3188x over previous
"""MoE (8 experts, top-2) Trainium2 kernel.

Strategy (per spec sharding_hint): expert parallelism. The host computes the
(cheap) router — logits, softmax, top-2, renormalized combine weights — and
dispatches each token to the cores owning its two experts ("all-to-all token
dispatch by top-k expert id" done at the sharding step, since kernel() holds
the full inputs host-side). Core e runs the expert-e FFN over its gathered
tokens, capacity-padded so all 8 cores run one SPMD program:

    Y = W2[e]^T @ gelu(W1[e]^T @ XT + b1[e])         (feature-major layouts)

Everything is bf16 on the PE (1 cycle/row) with fp32 PSUM accumulation.
Both weight matrices stay resident in SBUF for the whole kernel; the
gelu intermediate HT lives in SBUF half-buffers (never round-trips to
DRAM). Per 512-token tile: stage 1 fills HT (32 i-chunks x 8 k-matmuls),
stage 2 accumulates all 32 i-chunks into 8 output psum banks and drains
Y (bf16) to DRAM. Capacity is 128-granular (4224 for the seed-0 max
expert count of 4181), the DMA lead-in before the first matmul is 3
descriptors-light transfers, and drain copies alternate DVE/ACT. The
host then scatter-adds  (Y + b2[e]) * combine  into the full output.
"""

import os
import sys

import numpy as np

for _p in ("/opt/trn_rl_repo", "/root/.axon_site/_ro/trn_rl_repo"):
    if os.path.isdir(_p) and _p not in sys.path:
        sys.path.insert(0, _p)

NUM_EXPERTS = 8
TOP_K = 2
B, S, H, I = 4, 4096, 1024, 4096
T = B * S
P = 128
NT = 512           # max token tile = moving free dim (fp32 psum bank limit)
C_DEFAULT = 4224   # capacity per expert (seed-0 max count 4181), mult of 128

KH = H // P        # 8 contraction chunks for stage 1
KI = I // P        # 32 contraction chunks for stage 2
OB = H // P        # 8 output chunks for stage 2
HB = KI // 2       # 16 ht chunks per SBUF half-buffer

_built = {}        # (C, reps) -> nc


def _token_tiles(C):
    """Split C into tiles of 512 plus at most one trailing 128/256/384."""
    assert C % 128 == 0
    tiles, off = [], 0
    while C - off >= 512:
        tiles.append((off, 512))
        off += 512
    if C - off:
        tiles.append((off, C - off))
        off = C
    return tiles


def _build(C, reps=1):
    import concourse.bacc as bacc
    import concourse.mybir as mybir
    import concourse.tile as tile
    from concourse._compat import get_trn_type

    f32 = mybir.dt.float32
    bf16 = mybir.dt.bfloat16
    GELU = mybir.ActivationFunctionType.Gelu

    nc = bacc.Bacc(
        get_trn_type() or "TRN2",
        target_bir_lowering=False,
        debug=False,
        enable_asserts=False,
    )
    xt = nc.dram_tensor("xt", [H, C], bf16, kind="ExternalInput").ap()
    w1 = nc.dram_tensor("w1", [H, I], bf16, kind="ExternalInput").ap()
    # b1 arrives pre-transposed [P, KI] so its DMA is 128 contiguous rows
    b1 = nc.dram_tensor("b1", [P, KI], f32, kind="ExternalInput").ap()
    w2 = nc.dram_tensor("w2", [I, H], bf16, kind="ExternalInput").ap()
    y = nc.dram_tensor("y", [H, C], bf16, kind="ExternalOutput").ap()

    tiles = _token_tiles(C)

    with tile.TileContext(nc) as tc:
        with (
            tc.tile_pool(name="bias", bufs=1) as bpool,
            tc.tile_pool(name="w1p", bufs=1) as w1p,
            tc.tile_pool(name="w2p", bufs=1) as w2p,
            tc.tile_pool(name="xp", bufs=3) as xp,
            tc.tile_pool(name="htp", bufs=2) as htp,
            tc.tile_pool(name="yp", bufs=2) as yp,
            tc.tile_pool(name="psp", bufs=8, space="PSUM") as psp,
        ):
            b1sb = bpool.tile([P, KI], f32)
            w1r = w1.rearrange("(ko p) i -> p ko i", p=P)
            w2r = w2.rearrange("(ko p) o -> p ko o", p=P)

            for rep in range(reps):
                # Each weight matrix is split into two pool tiles so the
                # next rep's reload of one half can start while this rep is
                # still reading the other half (staggered, never serial).
                w1h = [w1p.tile([P, KH, I // 2], bf16, tag=f"w1{h}",
                                name=f"w1_{rep}_{h}") for h in range(2)]
                w2h = [w2p.tile([P, KI // 2, H], bf16, tag=f"w2{h}",
                                name=f"w2_{rep}_{h}") for h in range(2)]

                def _load_x(t, toff, tsz, ways, rep=rep):
                    xst = xp.tile([P, KH, tsz], bf16, tag="x",
                                  name=f"x_{rep}_{t}")
                    kw = KH // ways
                    for s in range(ways):
                        nc.sync.dma_start(
                            xst[:, s * kw:(s + 1) * kw],
                            xt[s * kw * P:(s + 1) * kw * P,
                               toff:toff + tsz].rearrange(
                                "(ko p) n -> p ko n", p=P),
                        )
                    return xst

                # Lead-in: x(t0) split 8 ways and w1's first i-chunk split in
                # two, so the PE's first matmul waits on ~1.25MB spread over
                # many queues; then the rest of w1 (i-column order, matching
                # stage-1 consumption) and w2 (k-chunk order for stage 2).
                # Lead-in: the PE's first matmul only needs w1 i-cols 0:128
                # and the first half of x(t0); keep the gating chain to 3
                # dma_starts (SP issues one per ~565ns), then stream the rest
                # in consumption order.
                IH = I // 2
                nc.sync.dma_start(w1h[0][:, :, 0:128], w1r[:, :, 0:128])
                xs0 = _load_x(0, tiles[0][0], tiles[0][1], ways=2)
                nc.sync.dma_start(b1sb[:], b1)
                for lo, hi in ((128, 640), (640, 1152), (1152, 1664),
                               (1664, 2048)):
                    nc.sync.dma_start(w1h[0][:, :, lo:hi], w1r[:, :, lo:hi])
                for lo, hi in ((0, 512), (512, 1024), (1024, 1536),
                               (1536, 2048)):
                    nc.sync.dma_start(
                        w1h[1][:, :, lo:hi], w1r[:, :, IH + lo:IH + hi]
                    )
                cw = 4
                for h in range(2):
                    for c in range(HB // cw):
                        nc.sync.dma_start(
                            w2h[h][:, c * cw:(c + 1) * cw],
                            w2r[:, h * HB + c * cw:h * HB + (c + 1) * cw],
                        )

                for t, (toff, tsz) in enumerate(tiles):
                    xs = xs0 if t == 0 else _load_x(t, toff, tsz, ways=2)

                    # -- Stage 1: HT[i,t] = gelu(W1^T XT + b1), 2 half-bufs
                    hts = []
                    for hb in range(2):
                        htt = htp.tile([P, HB, tsz], bf16, tag="ht",
                                       name=f"ht_{rep}_{t}_{hb}")
                        for ibl in range(HB):
                            ib = hb * HB + ibl
                            ps = psp.tile([P, tsz], f32, tag="ps",
                                          name=f"ps1_{rep}_{t}_{ib}")
                            ibl2 = ib - hb * HB
                            for k in range(KH):
                                nc.tensor.matmul(
                                    ps[:],
                                    lhsT=w1h[hb][:, k,
                                                 ibl2 * P:(ibl2 + 1) * P],
                                    rhs=xs[:, k],
                                    start=(k == 0),
                                    stop=(k == KH - 1),
                                )
                            nc.scalar.activation(
                                htt[:, ibl], ps[:], GELU,
                                bias=b1sb[:, ib:ib + 1],
                            )
                        hts.append(htt)

                    # -- Stage 2: Y[o,t] = sum_i W2[i,o] HT[i,t], all 32
                    # i-chunks accumulated in 8 psum banks (k-outer order so
                    # each ht chunk is consumed once, freeing it early).
                    pss = [psp.tile([P, tsz], f32, tag="ps",
                                    name=f"ps2_{rep}_{t}_{ob}")
                           for ob in range(OB)]
                    for k in range(KI):
                        hb, kk = divmod(k, HB)
                        for ob in range(OB):
                            nc.tensor.matmul(
                                pss[ob][:],
                                lhsT=w2h[hb][:, kk, ob * P:(ob + 1) * P],
                                rhs=hts[hb][:, kk],
                                start=(k == 0),
                                stop=(k == KI - 1),
                            )
                    # Drain: alternate DVE/ACT so the 8 psum->sbuf copies run
                    # in parallel pairs; store y in two half DMAs so the
                    # first can fly while the second half still copies.
                    ys = yp.tile([P, OB, tsz], bf16, tag="y",
                                 name=f"y_{rep}_{t}")
                    for ob in range(OB):
                        if ob % 2 == 0:
                            nc.vector.tensor_copy(ys[:, ob], pss[ob][:])
                        else:
                            nc.scalar.copy(ys[:, ob], pss[ob][:])
                        if ob % 4 == 3:
                            g = ob - 3
                            nc.sync.dma_start(
                                y[g * P:(g + 4) * P,
                                  toff:toff + tsz].rearrange(
                                    "(ob p) n -> p ob n", p=P),
                                ys[:, g:g + 4],
                            )
    nc.finalize()
    return nc


def _routing(hidden, router_w, router_b):
    """Top-2 routing, bit-matching the jax reference on CPU."""
    import jax
    import jax.numpy as jnp

    cpu = jax.local_devices(backend="cpu")[0]
    with jax.default_device(cpu):
        logits = jnp.einsum("bsh,he->bse", jnp.asarray(hidden),
                            jnp.asarray(router_w)) + jnp.asarray(router_b)
        probs = jax.nn.softmax(logits, axis=-1)
        tkp, tki = jax.lax.top_k(probs, TOP_K)
        tkp = tkp / jnp.sum(tkp, axis=-1, keepdims=True)
        tkp_np = np.asarray(tkp).reshape(T, TOP_K)
        tki_np = np.asarray(tki).reshape(T, TOP_K)
    return tkp_np, tki_np


def _prepare(hidden_states, w1, b1, w2, b2, router_w, router_b):
    """Host-side routing + dispatch: per-core in_maps and scatter metadata."""
    hidden_states = np.ascontiguousarray(hidden_states, dtype=np.float32)
    w1 = np.ascontiguousarray(w1, dtype=np.float32)
    b1 = np.ascontiguousarray(b1, dtype=np.float32)
    w2 = np.ascontiguousarray(w2, dtype=np.float32)
    b2 = np.ascontiguousarray(b2, dtype=np.float32)

    import ml_dtypes

    bf = ml_dtypes.bfloat16
    w1_bf = w1.astype(bf)
    w2_bf = w2.astype(bf)
    tkp, tki = _routing(hidden_states, router_w, router_b)
    x = hidden_states.reshape(T, H)

    idx_e, prob_e = [], []
    for e in range(NUM_EXPERTS):
        hit = tki == e                       # [T, 2] bool
        idx = np.nonzero(hit.any(axis=1))[0]
        pe = np.where(hit[idx, 0], tkp[idx, 0], tkp[idx, 1]).astype(np.float32)
        idx_e.append(idx)
        prob_e.append(pe)

    maxn = max(len(ix) for ix in idx_e)
    C = C_DEFAULT if maxn <= C_DEFAULT else ((maxn + 127) // 128) * 128

    in_maps = []
    for e in range(NUM_EXPERTS):
        ix = idx_e[e]
        xt = np.zeros((H, C), dtype=bf)
        xt[:, :len(ix)] = x[ix].T.astype(bf)
        in_maps.append({
            "xt": xt,
            "w1": w1_bf[e],
            "b1": np.ascontiguousarray(b1[e].reshape(KI, P).T),
            "w2": w2_bf[e],
        })
    return in_maps, C, idx_e, prob_e, b2


def kernel(hidden_states, w1, b1, w2, b2, router_w, router_b):
    from concourse import bass_utils

    in_maps, C, idx_e, prob_e, b2 = _prepare(
        hidden_states, w1, b1, w2, b2, router_w, router_b
    )
    if (C, 1) not in _built:
        _built[(C, 1)] = _build(C)
    nc = _built[(C, 1)]

    res = bass_utils.run_bass_kernel_spmd(
        nc, in_maps, core_ids=list(range(NUM_EXPERTS))
    ).results

    out = np.zeros((T, H), dtype=np.float32)
    for e in range(NUM_EXPERTS):
        ix = idx_e[e]
        ye = np.asarray(res[e]["y"][:, :len(ix)].T, dtype=np.float32)
        out[ix] += (ye + b2[e]) * prob_e[e][:, None]
    return out.reshape(B, S, H)


# revision 5
# speedup vs baseline: 1.6023x; 1.2150x over previous
"""MoE (8 experts, top-2) Trainium2 kernel.

Strategy (per spec sharding_hint): expert parallelism. The host computes the
(cheap) router — logits, softmax, top-2, renormalized combine weights — and
dispatches each token to the cores owning its two experts ("all-to-all token
dispatch by top-k expert id" done at the sharding step, since kernel() holds
the full inputs host-side). Core e runs the expert-e FFN over its gathered
tokens, capacity-padded so all 8 cores run one SPMD program:

    Y = W2[e]^T @ gelu(W1[e]^T @ XT + b1[e])         (feature-major layouts)

Everything is bf16 on the PE (1 cycle/row) with fp32 PSUM accumulation.
Both weight matrices stay resident in SBUF for the whole kernel; the
gelu intermediate HT lives in SBUF half-buffers (never round-trips to
DRAM). Per 512-token tile: stage 1 fills HT (32 i-chunks x 8 k-matmuls),
stage 2 accumulates all 32 i-chunks into 8 output psum banks and drains
Y to DRAM. The host then scatter-adds  (Y + b2[e]) * combine  into the
full output.
"""

import os
import sys

import numpy as np

for _p in ("/opt/trn_rl_repo", "/root/.axon_site/_ro/trn_rl_repo"):
    if os.path.isdir(_p) and _p not in sys.path:
        sys.path.insert(0, _p)

NUM_EXPERTS = 8
TOP_K = 2
B, S, H, I = 4, 4096, 1024, 4096
T = B * S
P = 128
NT = 512           # max token tile = moving free dim (fp32 psum bank limit)
C_DEFAULT = 4224   # capacity per expert (seed-0 max count 4181), mult of 128

KH = H // P        # 8 contraction chunks for stage 1
KI = I // P        # 32 contraction chunks for stage 2
OB = H // P        # 8 output chunks for stage 2
HB = KI // 2       # 16 ht chunks per SBUF half-buffer

_built = {}        # (C, reps) -> nc


def _token_tiles(C):
    """Split C into tiles of 512 plus at most one trailing 128/256/384."""
    assert C % 128 == 0
    tiles, off = [], 0
    while C - off >= 512:
        tiles.append((off, 512))
        off += 512
    if C - off:
        tiles.append((off, C - off))
        off = C
    return tiles


def _build(C, reps=1):
    import concourse.bacc as bacc
    import concourse.mybir as mybir
    import concourse.tile as tile
    from concourse._compat import get_trn_type

    f32 = mybir.dt.float32
    bf16 = mybir.dt.bfloat16
    GELU = mybir.ActivationFunctionType.Gelu

    nc = bacc.Bacc(
        get_trn_type() or "TRN2",
        target_bir_lowering=False,
        debug=False,
        enable_asserts=False,
    )
    xt = nc.dram_tensor("xt", [H, C], bf16, kind="ExternalInput").ap()
    w1 = nc.dram_tensor("w1", [H, I], bf16, kind="ExternalInput").ap()
    # b1 arrives pre-transposed [P, KI] so its DMA is 128 contiguous rows
    b1 = nc.dram_tensor("b1", [P, KI], f32, kind="ExternalInput").ap()
    w2 = nc.dram_tensor("w2", [I, H], bf16, kind="ExternalInput").ap()
    y = nc.dram_tensor("y", [H, C], bf16, kind="ExternalOutput").ap()

    tiles = _token_tiles(C)

    with tile.TileContext(nc) as tc:
        with (
            tc.tile_pool(name="bias", bufs=1) as bpool,
            tc.tile_pool(name="w1p", bufs=1) as w1p,
            tc.tile_pool(name="w2p", bufs=1) as w2p,
            tc.tile_pool(name="xp", bufs=3) as xp,
            tc.tile_pool(name="htp", bufs=2) as htp,
            tc.tile_pool(name="yp", bufs=2) as yp,
            tc.tile_pool(name="psp", bufs=8, space="PSUM") as psp,
        ):
            b1sb = bpool.tile([P, KI], f32)
            w1r = w1.rearrange("(ko p) i -> p ko i", p=P)
            w2r = w2.rearrange("(ko p) o -> p ko o", p=P)

            for rep in range(reps):
                # Each weight matrix is split into two pool tiles so the
                # next rep's reload of one half can start while this rep is
                # still reading the other half (staggered, never serial).
                w1h = [w1p.tile([P, KH, I // 2], bf16, tag=f"w1{h}",
                                name=f"w1_{rep}_{h}") for h in range(2)]
                w2h = [w2p.tile([P, KI // 2, H], bf16, tag=f"w2{h}",
                                name=f"w2_{rep}_{h}") for h in range(2)]

                def _load_x(t, toff, tsz, ways, rep=rep):
                    xst = xp.tile([P, KH, tsz], bf16, tag="x",
                                  name=f"x_{rep}_{t}")
                    kw = KH // ways
                    for s in range(ways):
                        nc.sync.dma_start(
                            xst[:, s * kw:(s + 1) * kw],
                            xt[s * kw * P:(s + 1) * kw * P,
                               toff:toff + tsz].rearrange(
                                "(ko p) n -> p ko n", p=P),
                        )
                    return xst

                # Lead-in: x(t0) split 8 ways and w1's first i-chunk split in
                # two, so the PE's first matmul waits on ~1.25MB spread over
                # many queues; then the rest of w1 (i-column order, matching
                # stage-1 consumption) and w2 (k-chunk order for stage 2).
                # Lead-in: the PE's first matmul only needs w1 i-cols 0:128
                # and x(t0) k-chunks 0-1; issue those three dma_starts first
                # (SP issues one per ~565ns), then stream the rest in
                # consumption order.
                IH = I // 2
                nc.sync.dma_start(w1h[0][:, :, 0:128], w1r[:, :, 0:128])
                xs0 = _load_x(0, tiles[0][0], tiles[0][1], ways=2)
                nc.sync.dma_start(b1sb[:], b1)
                for lo, hi in ((128, 640), (640, 1152), (1152, 1664),
                               (1664, 2048)):
                    nc.sync.dma_start(w1h[0][:, :, lo:hi], w1r[:, :, lo:hi])
                for lo, hi in ((0, 512), (512, 1024), (1024, 1536),
                               (1536, 2048)):
                    nc.sync.dma_start(
                        w1h[1][:, :, lo:hi], w1r[:, :, IH + lo:IH + hi]
                    )
                cw = 4
                for h in range(2):
                    for c in range(HB // cw):
                        nc.sync.dma_start(
                            w2h[h][:, c * cw:(c + 1) * cw],
                            w2r[:, h * HB + c * cw:h * HB + (c + 1) * cw],
                        )

                xs_tiles = {0: xs0}
                for t, (toff, tsz) in enumerate(tiles):
                    # Prefetch x(t+1) now, BEFORE this tile's compute and y
                    # stores are emitted, so its dma_start sits ahead of them
                    # in the SP queue rings.
                    if t + 1 < len(tiles):
                        nt_off, nt_sz = tiles[t + 1]
                        xs_tiles[t + 1] = _load_x(t + 1, nt_off, nt_sz,
                                                  ways=2)
                    xs = xs_tiles.pop(t)

                    # -- Stage 1: HT[i,t] = gelu(W1^T XT + b1), 2 half-bufs
                    hts = []
                    for hb in range(2):
                        htt = htp.tile([P, HB, tsz], bf16, tag="ht",
                                       name=f"ht_{rep}_{t}_{hb}")
                        for ibl in range(HB):
                            ib = hb * HB + ibl
                            ps = psp.tile([P, tsz], f32, tag="ps",
                                          name=f"ps1_{rep}_{t}_{ib}")
                            ibl2 = ib - hb * HB
                            for k in range(KH):
                                nc.tensor.matmul(
                                    ps[:],
                                    lhsT=w1h[hb][:, k,
                                                 ibl2 * P:(ibl2 + 1) * P],
                                    rhs=xs[:, k],
                                    start=(k == 0),
                                    stop=(k == KH - 1),
                                )
                            nc.scalar.activation(
                                htt[:, ibl], ps[:], GELU,
                                bias=b1sb[:, ib:ib + 1],
                            )
                        hts.append(htt)

                    # -- Stage 2: Y[o,t] = sum_i W2[i,o] HT[i,t], all 32
                    # i-chunks accumulated in 8 psum banks (k-outer order so
                    # each ht chunk is consumed once, freeing it early).
                    pss = [psp.tile([P, tsz], f32, tag="ps",
                                    name=f"ps2_{rep}_{t}_{ob}")
                           for ob in range(OB)]
                    for k in range(KI):
                        hb, kk = divmod(k, HB)
                        for ob in range(OB):
                            nc.tensor.matmul(
                                pss[ob][:],
                                lhsT=w2h[hb][:, kk, ob * P:(ob + 1) * P],
                                rhs=hts[hb][:, kk],
                                start=(k == 0),
                                stop=(k == KI - 1),
                            )
                    # Drain: alternate DVE/ACT so the 8 psum->sbuf copies run
                    # in parallel pairs; store y in two half DMAs so the
                    # first can fly while the second half still copies.
                    ys = yp.tile([P, OB, tsz], bf16, tag="y",
                                 name=f"y_{rep}_{t}")
                    for ob in range(OB):
                        if ob % 2 == 0:
                            nc.vector.tensor_copy(ys[:, ob], pss[ob][:])
                        else:
                            nc.scalar.copy(ys[:, ob], pss[ob][:])
                        if ob % 4 == 3:
                            g = ob - 3
                            nc.sync.dma_start(
                                y[g * P:(g + 4) * P,
                                  toff:toff + tsz].rearrange(
                                    "(ob p) n -> p ob n", p=P),
                                ys[:, g:g + 4],
                            )
    nc.finalize()
    return nc


def _routing(hidden, router_w, router_b):
    """Top-2 routing, bit-matching the jax reference on CPU."""
    import jax
    import jax.numpy as jnp

    cpu = jax.local_devices(backend="cpu")[0]
    with jax.default_device(cpu):
        logits = jnp.einsum("bsh,he->bse", jnp.asarray(hidden),
                            jnp.asarray(router_w)) + jnp.asarray(router_b)
        probs = jax.nn.softmax(logits, axis=-1)
        tkp, tki = jax.lax.top_k(probs, TOP_K)
        tkp = tkp / jnp.sum(tkp, axis=-1, keepdims=True)
        tkp_np = np.asarray(tkp).reshape(T, TOP_K)
        tki_np = np.asarray(tki).reshape(T, TOP_K)
    return tkp_np, tki_np


def _prepare(hidden_states, w1, b1, w2, b2, router_w, router_b):
    """Host-side routing + dispatch: per-core in_maps and scatter metadata."""
    hidden_states = np.ascontiguousarray(hidden_states, dtype=np.float32)
    w1 = np.ascontiguousarray(w1, dtype=np.float32)
    b1 = np.ascontiguousarray(b1, dtype=np.float32)
    w2 = np.ascontiguousarray(w2, dtype=np.float32)
    b2 = np.ascontiguousarray(b2, dtype=np.float32)

    import ml_dtypes

    bf = ml_dtypes.bfloat16
    w1_bf = w1.astype(bf)
    w2_bf = w2.astype(bf)
    tkp, tki = _routing(hidden_states, router_w, router_b)
    x = hidden_states.reshape(T, H)

    idx_e, prob_e = [], []
    for e in range(NUM_EXPERTS):
        hit = tki == e                       # [T, 2] bool
        idx = np.nonzero(hit.any(axis=1))[0]
        pe = np.where(hit[idx, 0], tkp[idx, 0], tkp[idx, 1]).astype(np.float32)
        idx_e.append(idx)
        prob_e.append(pe)

    maxn = max(len(ix) for ix in idx_e)
    C = C_DEFAULT if maxn <= C_DEFAULT else ((maxn + 127) // 128) * 128

    in_maps = []
    for e in range(NUM_EXPERTS):
        ix = idx_e[e]
        xt = np.zeros((H, C), dtype=bf)
        xt[:, :len(ix)] = x[ix].T.astype(bf)
        in_maps.append({
            "xt": xt,
            "w1": w1_bf[e],
            "b1": np.ascontiguousarray(b1[e].reshape(KI, P).T),
            "w2": w2_bf[e],
        })
    return in_maps, C, idx_e, prob_e, b2


def kernel(hidden_states, w1, b1, w2, b2, router_w, router_b):
    from concourse import bass_utils

    in_maps, C, idx_e, prob_e, b2 = _prepare(
        hidden_states, w1, b1, w2, b2, router_w, router_b
    )
    if (C, 1) not in _built:
        _built[(C, 1)] = _build(C)
    nc = _built[(C, 1)]

    res = bass_utils.run_bass_kernel_spmd(
        nc, in_maps, core_ids=list(range(NUM_EXPERTS))
    ).results

    out = np.zeros((T, H), dtype=np.float32)
    for e in range(NUM_EXPERTS):
        ix = idx_e[e]
        ye = np.asarray(res[e]["y"][:, :len(ix)].T, dtype=np.float32)
        out[ix] += (ye + b2[e]) * prob_e[e][:, None]
    return out.reshape(B, S, H)
